# revision 1
# baseline (speedup 1.0000x reference)
"""Trainium2 Bass kernel for nn_Conv2d_61615600828583 (LagrangeTrainer LUT conv, K=2).

Math: per table t with mask rows (s0,s1) and folded LUT coeffs (a,b,c,d):
  out[n,t] = 0.25*(a + b*x0 + c*x1 + d*x0*x1),  x_k = z[s_k, n]
where z is the (144, 1024) im2col tensor of shifted padded input channels.
Final conv output: out[b,oc,p] = A[oc] + sum_s Wlin[oc,s] z[s,p]
                                + sum_{t in oc} 0.25*d_t z[s0_t,p] z[s1_t,p].

Device strategy (per core = one batch, 8 cores data-parallel):
  - build z in fp16 in HBM (9 shifted cast-DMAs of the padded image)
  - dma_gather z rows into lane-aligned fp16 tiles (pair-deduped, cover-set
    lane assignment so the in0 operand is one fixed tile)
  - DVE fp16 tensor_mul for the 3724 distinct pair products
  - PE accumulates d-weighted reduction + linear conv into PSUM (fp32)
  - ACT evacuates PSUM with per-partition bias A, DMA out
"""
import numpy as np

B, IC, H, W = 8, 16, 32, 32
OC, KH, KW = 32, 3, 3
PAD, K = 1, 2
OH, OW = H, W
TABLES = OC * IC * KH * KW     # 4608
S = IC * KH * KW               # 144
NPX = OH * OW                  # 1024
NW = 16                        # |W| cover complement
IMG_W = (H + 2 * PAD)          # 34
IMG_LEN = IMG_W * IMG_W        # 1156


def _host_prep(w_luts, mask_c, mask_kh, mask_kw):
    w = np.asarray(w_luts, np.float64)
    mc = np.asarray(mask_c).astype(np.int64)
    mkh = np.asarray(mask_kh).astype(np.int64)
    mkw = np.asarray(mask_kw).astype(np.int64)

    a = w[:, 0] + w[:, 1] + w[:, 2] + w[:, 3]
    b = -w[:, 0] + w[:, 1] - w[:, 2] + w[:, 3]
    c = -w[:, 0] - w[:, 1] + w[:, 2] + w[:, 3]
    d = w[:, 0] - w[:, 1] - w[:, 2] + w[:, 3]

    s_idx = mc * 9 + mkh * 3 + mkw
    s0, s1 = s_idx[0::2], s_idx[1::2]
    oc_of_t = np.arange(TABLES) // (S)

    A = 0.25 * np.add.reduceat(a, np.arange(0, TABLES, S)).astype(np.float64)
    Wlin = np.zeros((OC, S))
    np.add.at(Wlin, (oc_of_t, s0), 0.25 * b)
    np.add.at(Wlin, (oc_of_t, s1), 0.25 * c)

    # dedup unordered pairs -> slot list with per-oc weight vectors
    pair_w = {}
    for t in range(TABLES):
        key = (min(s0[t], s1[t]), max(s0[t], s1[t]))
        v = pair_w.get(key)
        if v is None:
            v = np.zeros(OC)
            pair_w[key] = v
        v[oc_of_t[t]] += 0.25 * d[t]

    # cover set: W = 16 least-used s values (by pair degree), V = the rest
    deg = np.zeros(S, np.int64)
    for (u, v) in pair_w:
        deg[u] += 1
        if v != u:
            deg[v] += 1
    order = np.argsort(deg, kind="stable")
    Wset = sorted(order[:NW].tolist())
    Vset = sorted(order[NW:].tolist())
    lane_of = {s: i for i, s in enumerate(Vset)}

    normal, leftover = [], []
    for key, wv in pair_w.items():
        u, v = key
        lanes = [x for x in (lane_of.get(u), lane_of.get(v)) if x is not None]
        if not lanes:
            leftover.append((u, v, wv))
        else:
            normal.append((key, lanes, wv))
    assert len(leftover) <= 128

    # greedy lane balancing: forced pairs first, then min-load choice
    normal.sort(key=lambda e: len(e[1]))
    load = np.zeros(128, np.int64)
    lane_slots = [[] for _ in range(128)]   # per lane: list of (partner_row, wvec)
    for (u, v), lanes, wv in normal:
        ln = min(lanes, key=lambda x: load[x])
        partner = v if Vset[ln] == u else u
        lane_slots[ln].append((partner, wv))
        load[ln] += 1
    T_norm = int(load.max())

    # slot tables
    dw = np.zeros((128, T_norm + 1, OC), np.float16)   # +1 leftover tile
    part_rows = np.zeros((T_norm, 128), np.int64)      # in1 row per (tile, lane)
    for ln in range(128):
        for k, (partner, wv) in enumerate(lane_slots[ln]):
            part_rows[k, ln] = partner
            dw[ln, k, :] = wv.astype(np.float16)
    in0L = np.zeros(128, np.int64)
    in1L = np.zeros(128, np.int64)
    for i, (u, v, wv) in enumerate(leftover):
        in0L[i], in1L[i] = u, v
        dw[i, T_norm, :] = wv.astype(np.float16)

    # gather stream: tile0 = V rows, tile1 = W rows, tiles 2.. = partners,
    # then leftover in0, leftover in1
    gtiles = [np.array(Vset, np.int64),
              np.pad(np.array(Wset, np.int64), (0, 128 - NW))]
    for k in range(T_norm):
        gtiles.append(part_rows[k])
    gtiles.append(in0L)
    gtiles.append(in1L)
    Lfull = np.concatenate(gtiles)          # (ntg*128,)
    ntg = len(gtiles)

    # wrapped int16 idx tensor: within each gather chunk of tiles, the SWDGE
    # unwraps columns as (s p) over partitions 0..15. Chunks are defined in
    # _chunks(); wrap per chunk.
    chunks = _chunks(T_norm)
    idx_np = np.zeros((128, ntg * 8), np.int16)
    for (t0, t1) in chunks:
        flat = Lfull[t0 * 128:t1 * 128].astype(np.int16)
        cols = (t1 - t0) * 8
        blk = flat.reshape(cols, 16).T      # [16, cols]
        for g in range(8):
            idx_np[g * 16:(g + 1) * 16, t0 * 8:t0 * 8 + cols] = blk

    wlV = np.zeros((128, OC), np.float16)
    for i, s in enumerate(Vset):
        wlV[i] = Wlin[:, s].astype(np.float16)
    wlW = np.zeros((NW, OC), np.float16)
    for i, s in enumerate(Wset):
        wlW[i] = Wlin[:, s].astype(np.float16)

    return dict(
        dw=dw.reshape(128, (T_norm + 1) * OC),
        idx=idx_np,
        wlV=wlV,
        wlW=wlW,
        bias=A.astype(np.float32).reshape(OC, 1),
        T_norm=T_norm,
        ntg=ntg,
    )


def _chunks(T_norm):
    """gather chunks as (tile_start, tile_end) over the gather-tile stream."""
    ch = [(0, 2)]                       # V + W tiles
    t = 2
    while t < 2 + T_norm:
        e = min(t + 4, 2 + T_norm)
        ch.append((t, e))
        t = e
    ch.append((2 + T_norm, 4 + T_norm))  # leftover in0 + in1
    return ch


def _build_nc(T_norm, ntg):
    import concourse.bass as bass
    import concourse.mybir as mybir
    import concourse.tile as tile
    from concourse import bacc

    fp16 = mybir.dt.float16
    fp32 = mybir.dt.float32

    nc = bacc.Bacc("TRN2", target_bir_lowering=False, debug=False,
                   enable_asserts=False, num_devices=8)
    x_h = nc.dram_tensor("x", [IC, H, W], fp32, kind="ExternalInput")
    idx_h = nc.dram_tensor("idx", [128, ntg * 8], mybir.dt.int16, kind="ExternalInput")
    dw_h = nc.dram_tensor("dw", [128, (T_norm + 1) * OC], fp16, kind="ExternalInput")
    wlV_h = nc.dram_tensor("wlV", [128, OC], fp16, kind="ExternalInput")
    wlW_h = nc.dram_tensor("wlW", [NW, OC], fp16, kind="ExternalInput")
    bias_h = nc.dram_tensor("bias", [OC, 1], fp32, kind="ExternalInput")
    out_h = nc.dram_tensor("out", [OC, NPX], fp32, kind="ExternalOutput")

    chunks = _chunks(T_norm)

    with tile.TileContext(nc) as tc:
        with (
            tc.tile_pool(name="const", bufs=1) as constp,
            tc.tile_pool(name="gath", bufs=len(chunks)) as gathp,
            tc.tile_pool(name="prod", bufs=len(chunks)) as prodp,
            tc.tile_pool(name="psum", bufs=1, space="PSUM") as psump,
            tc.tile_pool(name="dram", bufs=1, space="DRAM") as dramp,
        ):
            idx_s = constp.tile([128, ntg * 8], mybir.dt.int16)
            dw_s = constp.tile([128, T_norm + 1, OC], fp16)
            wlV_s = constp.tile([128, OC], fp16)
            wlW_s = constp.tile([NW, OC], fp16)
            bias_s = constp.tile([OC, 1], fp32)
            img = constp.tile([IC, IMG_LEN], fp32)
            z16 = dramp.tile([S, NPX], fp16)
            in0x4 = constp.tile([128, 4, NPX], fp16)
            out_s = constp.tile([OC, NPX], fp32)
            psum = psump.tile([OC, NPX], fp32)

            nc.sync.dma_start(out=idx_s, in_=idx_h[:])
            nc.sync.dma_start(out=dw_s[:].rearrange("p a b -> p (a b)"), in_=dw_h[:])
            nc.sync.dma_start(out=wlV_s, in_=wlV_h[:])
            nc.sync.dma_start(out=wlW_s, in_=wlW_h[:])
            nc.sync.dma_start(out=bias_s, in_=bias_h[:])

            # padded image: zero borders then row DMA
            nc.vector.memset(img[:], 0.0)
            iv = img[:]
            dst = bass.AP(iv.tensor, iv.offset + IMG_W + 1,
                          [iv.ap[0], [IMG_W, H], [1, W]])
            nc.sync.dma_start(out=dst, in_=x_h[:])

            # z16 HBM: 9 shifted fp32->fp16 cast DMAs
            zv = z16[:]
            for kh in range(KH):
                for kw in range(KW):
                    src = bass.AP(iv.tensor, iv.offset + kh * IMG_W + kw,
                                  [iv.ap[0], [IMG_W, OH], [1, OW]])
                    dstz = bass.AP(zv.tensor, zv.offset + (kh * 3 + kw) * NPX,
                                   [[9 * NPX, IC], [OW, OH], [1, OW]])
                    nc.gpsimd.dma_start(out=dstz, in_=src)

            # gather chunks
            gts = []
            for (t0, t1) in chunks:
                g = gathp.tile([128, t1 - t0, NPX], fp16, tag="g")
                nidx = (t1 - t0) * 128
                nc.gpsimd.dma_gather(g[:], z16[:], idx_s[:, t0 * 8:t1 * 8],
                                     nidx, nidx, NPX)
                gts.append(g)

            g0 = gts[0]
            for k in range(4):
                nc.vector.tensor_copy(in0x4[:, k, :], g0[:, 0, :])

            # products
            prods = []           # (P_tile, local_j, slot_index) per normal tile
            for ci, (t0, t1) in enumerate(chunks[1:-1], start=1):
                n = chunks[ci][1] - chunks[ci][0]
                P = prodp.tile([128, 4, NPX], fp16, tag="P")
                nc.vector.tensor_mul(P[:, 0:n, :], in0x4[:, 0:n, :], gts[ci][:])
                for j in range(n):
                    prods.append((P, j, (t0 - 2) + j))
            gl = gts[-1]
            PL = prodp.tile([128, 1, NPX], fp16, tag="P")
            nc.vector.tensor_mul(PL[:, 0, :], gl[:, 0, :], gl[:, 1, :])

            # matmul accumulation: units = [linV, linW, slots..., leftover]
            units = []
            units.append((wlV_s[:], lambda c: g0[:, 0, c * 512:(c + 1) * 512]))
            units.append((wlW_s[:], lambda c: g0[0:NW, 1, c * 512:(c + 1) * 512]))
            for (P, j, sl) in prods:
                units.append((dw_s[:, sl, :],
                              lambda c, P=P, j=j: P[:, j, c * 512:(c + 1) * 512]))
            units.append((dw_s[:, T_norm, :],
                          lambda c: PL[:, 0, c * 512:(c + 1) * 512]))

            nu = len(units)
            for ui, (lhsT, rhsf) in enumerate(units):
                for c in range(2):
                    nc.tensor.matmul(psum[:, c * 512:(c + 1) * 512], lhsT, rhsf(c),
                                     start=(ui == 0), stop=(ui == nu - 1))

            nc.scalar.activation(out=out_s[:], in_=psum[:],
                                 func=mybir.ActivationFunctionType.Identity,
                                 bias=bias_s[:], scale=1.0)
            nc.sync.dma_start(out=out_h[:], in_=out_s[:])

    nc.compile()
    return nc


def kernel(**inputs) -> np.ndarray:
    from concourse.bass_utils import run_bass_kernel_spmd

    x = np.ascontiguousarray(np.asarray(inputs["input"], np.float32))
    prep = _host_prep(inputs["w_luts"], inputs["mask_c"],
                      inputs["mask_kh"], inputs["mask_kw"])
    nc = _build_nc(prep["T_norm"], prep["ntg"])

    const_map = {k: np.ascontiguousarray(prep[k]) for k in
                 ("idx", "dw", "wlV", "wlW", "bias")}
    in_maps = [dict(const_map, x=x[b]) for b in range(B)]
    res = run_bass_kernel_spmd(nc, in_maps, core_ids=list(range(B))).results
    out = np.stack([res[b]["out"].reshape(OC, OH, OW) for b in range(B)])
    return out.astype(np.float32)


if __name__ == "__main__":
    import oracle
    ins = oracle.get_inputs()
    got = kernel(**ins)
    exp = np.load("expected_cache.npy")
    rel = np.linalg.norm(got - exp) / np.linalg.norm(exp)
    print("Relative error:", rel)
